# revision 1
# baseline (speedup 1.0000x reference)
"""Cross-attention kernel for Trainium2, sharded over 8 NeuronCores.

Shards query rows across cores (1024 rows each); K/V work is replicated.
All matmuls run with fp16 operands (1 cycle/row on the PE, 4x faster than
fp32) accumulating in fp32 PSUM.  Raw q/k/v/W are cast fp32->fp16 during the
SWDGE load DMA and transposed on-chip with the DMA xbar (2-byte dtypes only),
so no compute engine spends cycles on layout.

Algebraic simplifications:
  - bk is dropped: Q @ (bk x 1)^T adds a row-constant to the scores, which
    softmax cancels.
  - softmax normalization and the post-softmax 1/sqrt(dk) scale are folded
    into one per-row multiply of the final PV product.
  - scores are stored as (chunkmax - s) in fp16; the error of the fp16
    representation vanishes for the entries near the max, which are the only
    ones softmax keeps.
"""
import sys

sys.path.insert(0, "/opt/trn_rl_repo")

import numpy as np  # noqa: E402
import concourse.bass as bass  # noqa: E402
import concourse.tile as tile  # noqa: E402
from concourse import mybir  # noqa: E402
from concourse import bass_utils  # noqa: E402
from contextlib import ExitStack  # noqa: E402

F16 = mybir.dt.float16
F32 = mybir.dt.float32
AF = mybir.ActivationFunctionType
AX = mybir.AxisListType
ALU = mybir.AluOpType

P = 128
D = 1024            # input dim
ND = D // P         # 8 d-chunks
C = 512             # dim_k
NCC = C // P        # 4 c-chunks
VD = 512            # dim_v
KEYS = 8192
NSLAB = KEYS // 512  # 16 slabs of 512 keys
RL = 1024           # rows per core
NB = RL // P        # 8 row blocks
NCORES = 8
NORM = float(1.0 / np.sqrt(np.float32(C)))

_ws_counter = [0]


def _split_multi_waits(nc):
    """This container's walrus accepts only ONE sync-wait per instruction.
    Move extra waits onto preceding same-engine EventSemaphore insts."""
    for f in nc.m.functions:
        for bb in f.blocks:
            il = bb.instructions
            if not any(
                inst.sync_info is not None and len(inst.sync_info.on_wait or ()) > 1
                for inst in il
            ):
                continue
            new = []
            for inst in il:
                si = inst.sync_info
                if si is not None and len(si.on_wait or ()) > 1:
                    waits = list(si.on_wait)
                    for w in waits[:-1]:
                        _ws_counter[0] += 1
                        new.append(
                            mybir.InstEventSemaphore(
                                name=f"I-ws{_ws_counter[0]}",
                                engine=inst.engine,
                                ins=[],
                                outs=[],
                                sync_info=mybir.SyncInfo(on_wait=[w], on_update=[]),
                            )
                        )
                    del si.on_wait[:-1]
                new.append(inst)
            bb.instructions = new


def _emit(nc, tc, aps):
    q_ap, k_ap, v_ap = aps["q"], aps["k"], aps["v"]
    wq_ap, wk_ap, wv_ap = aps["wq"], aps["wk"], aps["wv"]
    bq_ap, bv_ap = aps["bq"], aps["bv"]
    out_ap = aps["out"]

    with ExitStack() as top:
        const = top.enter_context(tc.tile_pool(name="const", bufs=1))
        big = top.enter_context(tc.tile_pool(name="big", bufs=1))

        ones = const.tile([1, P], F16, tag="ones")
        nc.vector.memset(ones[:], 1.0)
        bv16 = const.tile([1, VD], F16, tag="bv16")
        nc.gpsimd.dma_start(bv16[:], bv_ap[None, :])
        bqT = const.tile([P, NCC], F32, tag="bqT")
        nc.scalar.dma_start(bqT[:], bq_ap.rearrange("(j p) -> p j", p=P))

        # Long-lived operand stores (fp16):
        KT_sb = [big.tile([P, KEYS], F16, tag=f"KT{ci}", name=f"KT{ci}") for ci in range(NCC)]
        V_sb = [big.tile([P, 16 * VD], F16, tag=f"V{g}", name=f"V{g}") for g in range(4)]
        QT_sb = [big.tile([P, RL], F16, tag=f"QT{ci}", name=f"QT{ci}") for ci in range(NCC)]

        with ExitStack() as proj:
            wts = proj.enter_context(tc.tile_pool(name="wts", bufs=1))
            dram = proj.enter_context(tc.tile_pool(name="dram", bufs=1, space="DRAM"))
            xt = proj.enter_context(tc.tile_pool(name="xt", bufs=20))
            psp = proj.enter_context(tc.tile_pool(name="psp", bufs=4, space="PSUM"))

            # ---- stage fp16 copies in DRAM (SWDGE cast), then transpose with
            # ---- few, large xbar ops ([rows,128] DRAM -> [128,rows] SBUF) ----
            WT = {}
            for wname, w_ap in (("wq", wq_ap), ("wk", wk_ap), ("wv", wv_ap)):
                w16 = dram.tile([C, D], F16, tag=f"{wname}16", name=f"{wname}16")
                nc.gpsimd.dma_start(w16[:], w_ap[:])
                wt_d = [wts.tile([P, C], F16, tag=f"{wname}T{d}", name=f"{wname}T{d}") for d in range(ND)]
                for d in range(ND):
                    nc.sync.dma_start(wt_d[d][:], w16[:, d * P:(d + 1) * P],
                                      transpose=True)
                WT[wname] = wt_d

            # ---- Q projection: QT_sb[ci][:, rows] = Wq @ q^T + bq ----
            q16 = dram.tile([RL, D], F16, tag="q16", name="q16")
            nc.gpsimd.dma_start(q16[:], q_ap[:])
            qT = []
            for d in range(ND):
                t = xt.tile([P, RL], F16, tag="xt", name="qT_t")
                nc.sync.dma_start(t[:], q16[:, d * P:(d + 1) * P], transpose=True)
                qT.append(t)
            for ci in range(NCC):
                for rh in range(2):
                    ps = psp.tile([P, 512], F32, tag="psp")
                    for d in range(ND):
                        nc.tensor.matmul(
                            ps[:],
                            WT["wq"][d][:, ci * P:(ci + 1) * P],
                            qT[d][:, rh * 512:(rh + 1) * 512],
                            start=(d == 0),
                            stop=(d == ND - 1),
                        )
                    nc.scalar.activation(
                        QT_sb[ci][:, rh * 512:(rh + 1) * 512],
                        ps[:],
                        AF.Identity,
                        bias=bqT[:, ci:ci + 1],
                        scale=1.0,
                    )
            qT = None

            # ---- K/V projections, streamed by 1024-key group ----
            NG = KEYS // RL  # 8 groups of 1024 keys
            k16 = []
            v16 = []
            for g in range(NG):
                t = dram.tile([RL, D], F16, tag=f"k16_{g}", name=f"k16_{g}")
                nc.gpsimd.dma_start(t[:], k_ap[g * RL:(g + 1) * RL, :])
                k16.append(t)
                t = dram.tile([RL, D], F16, tag=f"v16_{g}", name=f"v16_{g}")
                nc.gpsimd.dma_start(t[:], v_ap[g * RL:(g + 1) * RL, :])
                v16.append(t)
            for g in range(NG):
                kT, vT = [], []
                for d in range(ND):
                    t = xt.tile([P, RL], F16, tag="xt", name="kT_t")
                    nc.sync.dma_start(t[:], k16[g][:, d * P:(d + 1) * P],
                                      transpose=True)
                    kT.append(t)
                    t = xt.tile([P, RL], F16, tag="xt", name="vT_t")
                    nc.sync.dma_start(t[:], v16[g][:, d * P:(d + 1) * P],
                                      transpose=True)
                    vT.append(t)
                # K^T[c, keys] (no bias: bk cancels in softmax)
                for sc in range(2):
                    s = g * 2 + sc
                    for ci in range(NCC):
                        ps = psp.tile([P, 512], F32, tag="psp")
                        for d in range(ND):
                            nc.tensor.matmul(
                                ps[:],
                                WT["wk"][d][:, ci * P:(ci + 1) * P],
                                kT[d][:, sc * 512:(sc + 1) * 512],
                                start=(d == 0),
                                stop=(d == ND - 1),
                            )
                        nc.scalar.copy(KT_sb[ci][:, s * 512:(s + 1) * 512], ps[:])
                # V[keys, v] with bv via rank-1 ones matmul
                for j in range(8):
                    ps = psp.tile([P, 512], F32, tag="psp")
                    nc.tensor.matmul(ps[:], ones[:], bv16[:], start=True, stop=False)
                    for d in range(ND):
                        nc.tensor.matmul(
                            ps[:],
                            vT[d][:, j * P:(j + 1) * P],
                            WT["wv"][d][:],
                            start=False,
                            stop=(d == ND - 1),
                        )
                    kc = g * 8 + j
                    nc.scalar.copy(
                        V_sb[kc // 16][:, (kc % 16) * VD:(kc % 16 + 1) * VD], ps[:]
                    )

        # ---- attention, one 128-row block at a time ----
        with ExitStack() as att:
            spool = att.enter_context(tc.tile_pool(name="spool", bufs=2))
            apool = att.enter_context(tc.tile_pool(name="apool", bufs=6))
            atp = att.enter_context(tc.tile_pool(name="atp", bufs=16))
            stat = att.enter_context(tc.tile_pool(name="stat", bufs=2))
            outp = att.enter_context(tc.tile_pool(name="outp", bufs=3))
            pss = att.enter_context(tc.tile_pool(name="pss", bufs=3, space="PSUM"))
            pso = att.enter_context(tc.tile_pool(name="pso", bufs=2, space="PSUM"))

            for b in range(NB):
                S = spool.tile([P, KEYS], F16, tag="S")  # stores chunkmax - s
                cm = stat.tile([P, NSLAB], F32, tag="cm")
                for sc in range(NSLAB):
                    ps = pss.tile([P, 512], F32, tag="pss")
                    for ci in range(NCC):
                        nc.tensor.matmul(
                            ps[:],
                            QT_sb[ci][:, b * P:(b + 1) * P],
                            KT_sb[ci][:, sc * 512:(sc + 1) * 512],
                            start=(ci == 0),
                            stop=(ci == NCC - 1),
                        )
                    nc.vector.reduce_max(cm[:, sc:sc + 1], ps[:], axis=AX.X)
                    # S' = chunkmax - s  (>= 0; near-max entries keep full precision)
                    nc.scalar.activation(
                        S[:, sc * 512:(sc + 1) * 512],
                        ps[:],
                        AF.Identity,
                        bias=cm[:, sc:sc + 1],
                        scale=-1.0,
                    )
                rm = stat.tile([P, 1], F32, tag="rm")
                nc.vector.reduce_max(rm[:], cm[:], axis=AX.X)
                bias_mat = stat.tile([P, NSLAB], F32, tag="bias")
                nc.vector.tensor_scalar(
                    bias_mat[:], cm[:], rm[:], None, op0=ALU.subtract
                )
                csum = stat.tile([P, NSLAB], F32, tag="csum")
                AT = []
                for sc in range(NSLAB):
                    A = apool.tile([P, 512], F16, tag="A")
                    # exp(-(S') + (cm - rm)) = exp(s - rowmax)
                    nc.scalar.activation(
                        A[:],
                        S[:, sc * 512:(sc + 1) * 512],
                        AF.Exp,
                        bias=bias_mat[:, sc:sc + 1],
                        scale=-1.0,
                        accum_out=csum[:, sc:sc + 1],
                    )
                    for jj in range(4):
                        t = atp.tile([P, P], F16, tag="AT", name="AT_t")
                        nc.sync.dma_start(
                            t[:], A[:, jj * P:(jj + 1) * P], transpose=True
                        )
                        AT.append(t)
                rs = stat.tile([P, 1], F32, tag="rs")
                nc.vector.reduce_sum(rs[:], csum[:], axis=AX.X)
                rinv = stat.tile([P, 1], F32, tag="rinv")
                nc.vector.reciprocal(rinv[:], rs[:])

                po = pso.tile([P, VD], F32, tag="pso")
                for kc in range(64):
                    nc.tensor.matmul(
                        po[:],
                        AT[kc][:],
                        V_sb[kc // 16][:, (kc % 16) * VD:(kc % 16 + 1) * VD],
                        start=(kc == 0),
                        stop=(kc == 63),
                    )
                out_sb = outp.tile([P, VD], F32, tag="out")
                nc.vector.tensor_scalar(
                    out_sb[:], po[:], rinv[:], NORM, op0=ALU.mult, op1=ALU.mult
                )
                nc.scalar.dma_start(out_ap[b * P:(b + 1) * P, :], out_sb[:])


_cached = {}


def _build():
    if "nc" in _cached:
        return _cached["nc"]
    nc = bass.Bass("TRN2", target_bir_lowering=False, debug=False)
    aps = {
        "q": nc.dram_tensor("q", [RL, D], F32, kind="ExternalInput").ap(),
        "k": nc.dram_tensor("k", [KEYS, D], F32, kind="ExternalInput").ap(),
        "v": nc.dram_tensor("v", [KEYS, D], F32, kind="ExternalInput").ap(),
        "wq": nc.dram_tensor("wq", [C, D], F32, kind="ExternalInput").ap(),
        "wk": nc.dram_tensor("wk", [C, D], F32, kind="ExternalInput").ap(),
        "wv": nc.dram_tensor("wv", [C, D], F32, kind="ExternalInput").ap(),
        "bq": nc.dram_tensor("bq", [C], F32, kind="ExternalInput").ap(),
        "bv": nc.dram_tensor("bv", [VD], F32, kind="ExternalInput").ap(),
        "out": nc.dram_tensor("out", [RL, VD], F32, kind="ExternalOutput").ap(),
    }
    with tile.TileContext(nc) as tc:
        _emit(nc, tc, aps)
    _split_multi_waits(nc)
    _cached["nc"] = nc
    return nc


def kernel(q, k, v, Wq, bq, Wk, bk, Wv, bv, _trace=False, _tmpdir=None):
    del bk  # provably cancels inside the softmax
    nc = _build()
    f32 = lambda a: np.ascontiguousarray(np.asarray(a, dtype=np.float32))
    q, k, v = f32(q), f32(k), f32(v)
    base = {
        "k": k, "v": v, "wq": f32(Wq), "wk": f32(Wk), "wv": f32(Wv),
        "bq": f32(bq), "bv": f32(bv),
    }
    in_maps = [
        dict(base, q=np.ascontiguousarray(q[c * RL:(c + 1) * RL]))
        for c in range(NCORES)
    ]
    res = bass_utils.run_bass_kernel_spmd(
        nc, in_maps, core_ids=list(range(NCORES)), trace=_trace, tmpdir=_tmpdir
    )
    out = np.concatenate([res.results[c]["out"] for c in range(NCORES)], axis=0)
    if _trace:
        kernel.last_results = res
    return out

